# revision 41
# baseline (speedup 1.0000x reference)
"""Trainium2 Bass kernel for nn_Attention (GQA + RoPE + sliding-window mask).

Sharding: tensor-parallel over heads across 8 cores. Each core gets 4 q heads
and exactly 1 kv head (32 q / 8 kv heads, GQA group = 4). The reference's
quirky output flatten ((H,S,D)->(H,D,S)->reshape(S, H*D)) makes the final
projection contract over (d-parity, sequence) instead of heads, so the final
output is row-sharded by head block: core c produces rows [256c, 256c+256) of
the (2048, 4096) result with NO collective at all.

Per-core pipeline (all on one NeuronCore, same program on all 8 = pure SPMD):
  phase 1: QKV projections (fp32r matmuls) + RoPE (+fold sqrt(scale) into the
           rope tables of both q and k) + PE transposes into [d, s] layouts.
  phase 2: per (head, 512-query-super): scores (fp32r), 2-pass masked softmax
           (DVE max / ACT fused exp+sum), PE-transpose P to [k, q] (bf16),
           PV matmul (bf16) -> A^T, transpose back, normalize.
  phase 3: final projection vs full wo (bf16), row slice out.
"""

import numpy as np
from contextlib import ExitStack

P = 128
D = 128  # head dim
NH = 4   # q heads per core
CORES = 8
NEG_THRESH = -1e8


def _dtypes():
    import concourse.mybir as mybir

    return mybir


def build_attention_nc(
    SEQ,
    DIM,
    plan,
    n_uniq,
    p_dt_name="bfloat16",
    wo_dt_name="bfloat16",
    proj_dt_name="bfloat16",
    proj_f32r=True,
    score_f32r=True,
    use_dma_t=True,
):
    """Build the per-core Bass program.

    plan: list over q-tiles i (SEQ//128 entries) of lists of (chunk_idx, uid)
          where uid == -1 means the 512-wide chunk needs no mask add, else the
          index into the maskb tensor. Chunks absent from the list are fully
          masked (skipped).
    """
    import concourse.bass as bass
    import concourse.bacc as bacc
    import concourse.mybir as mybir
    import concourse.tile as tile
    from concourse.masks import make_identity

    f32 = mybir.dt.float32
    f32r = mybir.dt.float32r
    P_DT = getattr(mybir.dt, p_dt_name)
    WO_DT = getattr(mybir.dt, wo_dt_name)
    PJ_DT = getattr(mybir.dt, proj_dt_name)
    pj_f32r = proj_f32r and proj_dt_name == "float32"

    ST = SEQ // P          # 16 s-tiles
    DD = DIM // P          # 32 contraction tiles
    KC = SEQ // 512        # 4 key chunks
    QS = SEQ // 512        # 4 query supers
    EW = NH * D            # 512 q-projection width
    JT = 2 * SEQ // P      # 32 j-tiles for final matmul
    MC = DIM // 512        # 8 output chunks
    ITILES = (NH * 64) // P  # 2 output row tiles
    assert NH == 4 and SEQ % 512 == 0 and DIM % 512 == 0

    def mm_cast(ap, use_r):
        return ap.bitcast(f32r) if use_r else ap

    nc = bacc.Bacc(trn_type="TRN2", debug=False, num_devices=CORES)

    # x pre-tiled on host: xT[p, st, t, si] = x[st*128+si, t*128+p] so each
    # streamed chunk is one DMA with 2KB contiguous per-partition runs
    xT = nc.dram_tensor("xT", [P, ST, DD, P], PJ_DT, kind="ExternalInput").ap()
    wT = nc.dram_tensor("wT", [DIM, EW + 2 * D], PJ_DT, kind="ExternalInput").ap()
    cs = nc.dram_tensor("cs", [SEQ, EW], f32, kind="ExternalInput").ap()
    mb = nc.dram_tensor(
        "maskb", [max(n_uniq, 1), P, 512], f32, kind="ExternalInput"
    ).ap()
    woT = nc.dram_tensor("woT", [2 * SEQ, DIM], WO_DT, kind="ExternalInput").ap()
    out = nc.dram_tensor("out", [NH * 64, DIM], f32, kind="ExternalOutput").ap()

    with tile.TileContext(nc) as tc, ExitStack() as ctx:
        const = ctx.enter_context(tc.tile_pool(name="const", bufs=1))
        idF = const.tile([P, P], f32)
        make_identity(nc, idF)
        idP = const.tile([P, P], P_DT)
        make_identity(nc, idP)
        zeros = const.tile([P, 512], f32)
        nc.vector.memset(zeros, 0.0)

        pers = ctx.enter_context(tc.tile_pool(name="pers", bufs=1))
        QTt = pers.tile([P, NH, ST * P], f32)   # [d, h, s]
        KTt = pers.tile([P, ST * P], f32)       # [d, s]
        Vt = pers.tile([P, ST, D], P_DT)        # [k(part), ktile, d]
        if n_uniq > 0:
            mbt = pers.tile([P, n_uniq, 512], f32)

        # ---------------- phase 1: projections + rope + layout ----------------
        with (
            tc.tile_pool(name="wpool", bufs=1) as wpool,
            tc.tile_pool(name="xpool", bufs=6) as xpool,
            tc.tile_pool(name="cspool", bufs=2) as cspool,
            tc.tile_pool(name="rpool", bufs=2) as rpool,
            tc.tile_pool(name="qps", bufs=2, space="PSUM") as qps,
            tc.tile_pool(name="kvps", bufs=2, space="PSUM") as kvps,
            tc.tile_pool(name="tps", bufs=2, space="PSUM") as tps,
            tc.tile_pool(name="t2ps", bufs=2, space="PSUM") as t2ps,
        ):
            XGW = min(8, DD)
            wTt = wpool.tile([P, DD, EW + 2 * D], PJ_DT)
            wTr = wT.rearrange("(t p) e -> p t e", p=P)

            XG = min(8, DD)  # dd-tiles per streamed x chunk
            NG = DD // XG
            xTr = xT
            # Interleave the weight-chunk loads with s-tile 0's x chunks so
            # the first matmuls start as soon as chunk 0 of each lands.
            st0_x = []
            for g in range(NG):
                xTt = xpool.tile([P, XG, P], PJ_DT, tag="xT")
                nc.sync.dma_start(
                    out=xTt, in_=xTr[:, 0, g * XG : (g + 1) * XG, :]
                )
                st0_x.append(xTt)
                gw = g % (DD // XGW)
                nc.sync.dma_start(
                    out=wTt[:, gw * XGW : (gw + 1) * XGW, :],
                    in_=wTr[:, gw * XGW : (gw + 1) * XGW, :],
                )
            for st in range(ST):
                cst = cspool.tile([P, EW], f32, tag="cs")
                nc.sync.dma_start(out=cst, in_=cs[st * P : (st + 1) * P, :])

                Qp = qps.tile([P, EW], f32, tag="Qp")
                KVp = kvps.tile([P, 2 * D], f32, tag="KVp")
                for g in range(DD // XG):
                    if st == 0:
                        xTt = st0_x[g]
                    else:
                        xTt = xpool.tile([P, XG, P], PJ_DT, tag="xT")
                        nc.sync.dma_start(
                            out=xTt,
                            in_=xTr[:, st, g * XG : (g + 1) * XG, :],
                        )
                    for tt in range(XG):
                        t = g * XG + tt
                        lhsT = mm_cast(xTt[:, tt, :], pj_f32r)
                        nc.tensor.matmul(
                            Qp,
                            lhsT,
                            mm_cast(wTt[:, t, 0:EW], pj_f32r),
                            start=(t == 0),
                            stop=(t == DD - 1),
                        )
                        nc.tensor.matmul(
                            KVp,
                            lhsT,
                            mm_cast(wTt[:, t, EW : EW + 2 * D], pj_f32r),
                            start=(t == 0),
                            stop=(t == DD - 1),
                        )

                # rope via strided even/odd halves (2-level APs only — 3-level
                # APs overflow the fixed ISA instruction encoding).
                # tensor_tensor_reduce instead of tensor_tensor: the plain TT
                # ISA struct has a single sync-wait slot and walrus codegen
                # rejects the PE+DMA double wait Tile emits here; the TTR/ISA
                # struct carries up to 8. accum outputs are dummies.
                def ttr_ew(out, in0, in1, op):
                    nc.vector.tensor_tensor(out=out, in0=in0, in1=in1, op=op)

                A_ = mybir.AluOpType
                HF = EW // 2  # 256: cos table width for q
                rq = rpool.tile([P, EW], f32, tag="rq")
                t1 = rpool.tile([P, HF], f32, tag="t1")
                t2 = rpool.tile([P, HF], f32, tag="t2")
                q_ev, q_od = Qp[:, 0:EW:2], Qp[:, 1:EW:2]
                cosr, sinr = cst[:, 0:HF], cst[:, HF : 2 * HF]
                ttr_ew(t1, q_ev, cosr, A_.mult)
                ttr_ew(t2, q_od, sinr, A_.mult)
                ttr_ew(rq[:, 0:EW:2], t1, t2, A_.subtract)
                ttr_ew(t1, q_ev, sinr, A_.mult)
                ttr_ew(t2, q_od, cosr, A_.mult)
                ttr_ew(rq[:, 1:EW:2], t1, t2, A_.add)

                rk = rpool.tile([P, D], f32, tag="rk")
                k_ev, k_od = KVp[:, 0:D:2], KVp[:, 1:D:2]
                cosk, sink = cst[:, 0 : D // 2], cst[:, HF : HF + D // 2]
                ttr_ew(t1[:, 0 : D // 2], k_ev, cosk, A_.mult)
                ttr_ew(t2[:, 0 : D // 2], k_od, sink, A_.mult)
                ttr_ew(rk[:, 0:D:2], t1[:, 0 : D // 2], t2[:, 0 : D // 2], A_.subtract)
                ttr_ew(t1[:, 0 : D // 2], k_ev, sink, A_.mult)
                ttr_ew(t2[:, 0 : D // 2], k_od, cosk, A_.mult)
                ttr_ew(rk[:, 1:D:2], t1[:, 0 : D // 2], t2[:, 0 : D // 2], A_.add)

                # V -> bf16 [k, d] layout (ACT copy, cast)
                nc.scalar.activation(
                    out=Vt[:, st, :],
                    in_=KVp[:, D : 2 * D],
                    func=mybir.ActivationFunctionType.Copy,
                )

                # transpose rq (per head) and rk into [d, s] layouts
                T1 = tps.tile([P, EW], f32, tag="T1")
                for h in range(NH):
                    nc.tensor.transpose(
                        T1[:, h * P : (h + 1) * P], rq[:, h * P : (h + 1) * P], idF
                    )
                # write as f32r so walrus accepts them as f32r matmul operands
                nc.vector.tensor_copy(
                    out=mm_cast(QTt[:, :, st * P : (st + 1) * P], score_f32r),
                    in_=T1.rearrange("p (h s) -> p h s", h=NH),
                )
                T2 = t2ps.tile([P, P], f32, tag="T2")
                nc.tensor.transpose(T2, rk, idF)
                nc.vector.tensor_copy(
                    out=mm_cast(KTt[:, st * P : (st + 1) * P], score_f32r), in_=T2
                )

        # ---------------- phase 2: attention ----------------
        if n_uniq > 0:
            nc.sync.dma_start(out=mbt, in_=mb.rearrange("u p m -> p u m"))
        apool = ctx.enter_context(tc.tile_pool(name="apool", bufs=1))
        # split by head-pair so phase 3's first row-tile can start once
        # heads 0-1 finish, overlapping the rest of phase 2
        Aall = [
            apool.tile([P, 2 * ST * D], P_DT, name=f"Aall{i}")
            for i in range(NH // 2)
        ]
        with (
            tc.tile_pool(name="ptsb", bufs=2) as ptsb,
            tc.tile_pool(name="spool", bufs=6) as spool,
            tc.tile_pool(name="ppool", bufs=4) as ppool,
            tc.tile_pool(name="stat", bufs=12) as stat,
            tc.tile_pool(name="atsb", bufs=3) as atsb,
            tc.tile_pool(name="sps", bufs=2, space="PSUM") as sps,
            tc.tile_pool(name="ptps", bufs=2, space="PSUM") as ptps,
            tc.tile_pool(name="atps", bufs=1, space="PSUM") as atps,
            tc.tile_pool(name="aps", bufs=1, space="PSUM") as aps,
            tc.tile_pool(name="wopool", bufs=2) as wopool,
            tc.tile_pool(name="osb", bufs=2) as osb,
            tc.tile_pool(name="ops", bufs=2, space="PSUM") as ops,
        ):
            for h in range(NH):
                for qs in range(QS):
                    PTt = ptsb.tile([P, ST, 512], P_DT, tag="PT")
                    kts_used = set()
                    recips = []
                    pt_written = set()
                    for qi in range(4):
                        i = 4 * qs + qi
                        row = plan[i]
                        if not row:
                            recips.append(None)
                            continue
                        pairs = [row[k : k + 2] for k in range(0, len(row), 2)]
                        stats = stat.tile([P, KC], f32, tag="stats")
                        ncols = 0
                        S_tiles = []
                        for pr in pairs:
                            W = 512 * len(pr)
                            S = sps.tile([P, 1024], f32, tag="S")
                            for k, (c, uid) in enumerate(pr):
                                sl = S[:, k * 512 : (k + 1) * 512]
                                nc.tensor.matmul(
                                    sl,
                                    mm_cast(
                                        QTt[:, h, i * P : (i + 1) * P], score_f32r
                                    ),
                                    mm_cast(
                                        KTt[:, c * 512 : (c + 1) * 512], score_f32r
                                    ),
                                    start=True,
                                    stop=True,
                                )
                                if uid >= 0:
                                    nc.vector.tensor_add(sl, sl, mbt[:, uid, :])
                                nc.vector.tensor_reduce(
                                    out=stats[:, ncols : ncols + 1],
                                    in_=sl,
                                    axis=mybir.AxisListType.X,
                                    op=mybir.AluOpType.max,
                                )
                                ncols += 1
                            S_tiles.append((S, pr))
                        negm = stat.tile([P, 1], f32, tag="negm")
                        nc.vector.tensor_reduce(
                            out=negm,
                            in_=stats[:, 0:ncols],
                            axis=mybir.AxisListType.X,
                            op=mybir.AluOpType.max,
                            negate=True,
                        )
                        sums = stat.tile([P, KC], f32, tag="sums")
                        for k, (Sk, pr) in enumerate(S_tiles):
                            W = 512 * len(pr)
                            Pt = ppool.tile([P, 1024], P_DT, tag="P")
                            nc.scalar.activation(
                                out=Pt[:, 0:W],
                                in_=Sk[:, 0:W],
                                func=mybir.ActivationFunctionType.Exp,
                                bias=negm,
                            )
                            nc.vector.tensor_reduce(
                                out=sums[:, k : k + 1],
                                in_=Pt[:, 0:W],
                                axis=mybir.AxisListType.X,
                                op=mybir.AluOpType.add,
                            )
                            # transpose P [q, k] -> PT [k, q]
                            for j, (c, uid) in enumerate(pr):
                                if use_dma_t:
                                    nc.sync.dma_start_transpose(
                                        out=PTt[
                                            :, 4 * c : 4 * c + 4, qi * P : (qi + 1) * P
                                        ],
                                        in_=Pt[:, j * 512 : (j + 1) * 512],
                                    )
                                else:
                                    PTp = ptps.tile([P, 512], P_DT, tag="PTp")
                                    for jj in range(4):
                                        nc.tensor.transpose(
                                            PTp[:, jj * P : (jj + 1) * P],
                                            Pt[:, j * 512 + jj * P : j * 512 + (jj + 1) * P],
                                            idP,
                                        )
                                    nc.vector.tensor_copy(
                                        out=PTt[:, 4 * c : 4 * c + 4, qi * P : (qi + 1) * P],
                                        in_=PTp.rearrange("p (kt q) -> p kt q", kt=4),
                                    )
                                for jj in range(4):
                                    kts_used.add(4 * c + jj)
                                    pt_written.add((4 * c + jj, qi))
                        denom = stat.tile([P, 1], f32, tag="denom")
                        nc.vector.tensor_reduce(
                            out=denom,
                            in_=sums[:, 0 : len(S_tiles)],
                            axis=mybir.AxisListType.X,
                            op=mybir.AluOpType.add,
                        )
                        recip = stat.tile([P, 1], f32, tag="recip")
                        nc.vector.reciprocal(recip, denom)
                        recips.append(recip)

                    # zero-fill PT holes (only for non-causal masks)
                    kts = sorted(kts_used)
                    for kt in kts:
                        for qi in range(4):
                            if (kt, qi) not in pt_written and recips[qi] is not None:
                                nc.vector.memset(
                                    PTt[:, kt, qi * P : (qi + 1) * P], 0.0
                                )
                            elif recips[qi] is None:
                                nc.vector.memset(
                                    PTt[:, kt, qi * P : (qi + 1) * P], 0.0
                                )

                    if not kts:
                        continue
                    # PV: A^T[d, q] accumulated over key tiles
                    At = atps.tile([P, 512], f32, tag="At")
                    for n, kt in enumerate(kts):
                        nc.tensor.matmul(
                            At,
                            Vt[:, kt, :],
                            PTt[:, kt, :],
                            start=(n == 0),
                            stop=(n == len(kts) - 1),
                        )
                    Atsb = atsb.tile([P, 512], P_DT, tag="Atsb")
                    nc.vector.tensor_copy(out=Atsb, in_=At)
                    Ap = aps.tile([P, 512], P_DT, tag="Ap")
                    for qi in range(4):
                        nc.tensor.transpose(
                            Ap[:, qi * P : (qi + 1) * P],
                            Atsb[:, qi * P : (qi + 1) * P],
                            idP,
                        )
                    # Aall layout: [sp, (t*2 + dd)*128 + hb*64 + p] so the final
                    # matmul's stationary slices are contiguous (walrus requires
                    # a single free dim on weight APs)
                    Ah = Aall[h // 2]
                    hb = h % 2
                    for qi in range(4):
                        i = 4 * qs + qi
                        # dview[sp, p, dd] == Ah[:, i*256 + dd*128 + hb*64 + p]
                        dview = Ah[:, i * 2 * P : (i + 1) * 2 * P].rearrange(
                            "a (dd j) -> a dd j", dd=2
                        )[:, :, hb * 64 : hb * 64 + 64].rearrange(
                            "a dd p -> a p dd"
                        )
                        if recips[qi] is None:
                            nc.vector.memset(dview, 0.0)
                            continue
                        nc.scalar.activation(
                            out=dview,
                            in_=Ap[:, qi * P : (qi + 1) * P].rearrange(
                                "a (p two) -> a p two", two=2
                            ),
                            func=mybir.ActivationFunctionType.Copy,
                            scale=recips[qi],
                        )

            # ---------------- phase 3: output projection ----------------
            for mc in range(MC):
                wot = wopool.tile([P, JT, 512], WO_DT, tag="wo")
                nc.sync.dma_start(
                    out=wot,
                    in_=woT[:, mc * 512 : (mc + 1) * 512].rearrange(
                        "(t p) m -> p t m", p=P
                    ),
                )
                for it in range(ITILES):
                    O = ops.tile([P, 512], f32, tag="O")
                    Av = Aall[it]
                    for jt in range(JT):
                        ddj, t = jt // ST, jt % ST
                        lhsT = Av[:, (t * 2 + ddj) * P : (t * 2 + ddj + 1) * P]
                        nc.tensor.matmul(
                            O,
                            lhsT,
                            wot[:, jt, :],
                            start=(jt == 0),
                            stop=(jt == JT - 1),
                        )
                    Ot = osb.tile([P, 512], f32, tag="Ot")
                    nc.scalar.activation(
                        out=Ot, in_=O, func=mybir.ActivationFunctionType.Copy
                    )
                    nc.sync.dma_start(
                        out=out[it * P : (it + 1) * P, mc * 512 : (mc + 1) * 512],
                        in_=Ot,
                    )

    # Bacc.compile() legalizes sync (>=2 waits split into EventSemaphore
    # instructions — this walrus caps every instruction at ONE sync wait)
    nc.compile()
    return nc


def analyze_mask(mask, SEQ):
    """Classify 128x512 mask blocks: skip / free / masked(dedup uid)."""
    ST = SEQ // P
    KC = SEQ // 512
    uniq = {}
    blocks = []
    plan = []
    for i in range(ST):
        row = []
        for c in range(KC):
            blk = mask[i * P : (i + 1) * P, c * 512 : (c + 1) * 512]
            if (blk <= NEG_THRESH).all():
                continue
            if not blk.any():
                row.append((c, -1))
            else:
                key = blk.tobytes()
                if key not in uniq:
                    uniq[key] = len(blocks)
                    blocks.append(np.ascontiguousarray(blk))
                row.append((c, uniq[key]))
        if not row:
            # fully masked query rows: keep all chunks so softmax matches
            # the reference's uniform distribution over -1e9 logits
            for c in range(KC):
                blk = mask[i * P : (i + 1) * P, c * 512 : (c + 1) * 512]
                key = blk.tobytes()
                if key not in uniq:
                    uniq[key] = len(blocks)
                    blocks.append(np.ascontiguousarray(blk))
                row.append((c, uniq[key]))
        plan.append(row)
    return plan, blocks


def make_rope_tables(cos_freq, sin_freq, SEQ, scale_quarter):
    """Build replicated [cos2 | sin2] tables with sqrt(SCALE) folded in.

    [cos_rep (SEQ, NH*64) | sin_rep (SEQ, NH*64)], sqrt(scale) folded in
    """
    cos_t = np.tile(np.asarray(cos_freq, np.float32) * scale_quarter, (1, NH))
    sin_t = np.tile(np.asarray(sin_freq, np.float32) * scale_quarter, (1, NH))
    return np.ascontiguousarray(
        np.concatenate([cos_t, sin_t], axis=1).astype(np.float32)
    )


_BUILD_CACHE = {}


def kernel(
    x,
    cos_freq,
    sin_freq,
    positions,
    mask,
    wq,
    wk,
    wv,
    wo,
    _trace=False,
):
    import sys

    if "/opt/trn_rl_repo" not in sys.path:
        sys.path.insert(0, "/opt/trn_rl_repo")
    from concourse.bass_utils import run_bass_kernel_spmd

    x = np.asarray(x, np.float32)
    mask = np.asarray(mask, np.float32)
    wq = np.asarray(wq, np.float32)
    wk = np.asarray(wk, np.float32)
    wv = np.asarray(wv, np.float32)
    wo = np.asarray(wo, np.float32)
    SEQ, DIM = x.shape
    assert wq.shape[0] == CORES * NH * D and wk.shape[0] == CORES * D
    assert 2 * SEQ == wq.shape[0], "flatten structure requires H*D == 2*SEQ"

    plan, blocks = analyze_mask(mask, SEQ)
    n_uniq = len(blocks)
    key = (SEQ, DIM, tuple(tuple(r) for r in plan))
    if key not in _BUILD_CACHE:
        _BUILD_CACHE[key] = build_attention_nc(SEQ, DIM, plan, n_uniq)
    nc = _BUILD_CACHE[key]

    import ml_dtypes

    bf16 = ml_dtypes.bfloat16
    scale_quarter = np.float32(D ** -0.25)
    cs = make_rope_tables(cos_freq, sin_freq, SEQ, scale_quarter)
    ST_, DD_ = SEQ // P, DIM // P
    xT = np.ascontiguousarray(
        x.reshape(ST_, P, DD_, P).transpose(3, 0, 2, 1)
    ).astype(bf16)
    woT = np.ascontiguousarray(wo.T).astype(bf16)
    if n_uniq:
        mbs = np.ascontiguousarray(np.stack(blocks, axis=0))
    else:
        mbs = np.zeros((1, P, 512), np.float32)

    in_maps = []
    for c in range(CORES):
        w_c = np.concatenate(
            [
                wq[c * NH * D : (c + 1) * NH * D],
                wk[c * D : (c + 1) * D],
                wv[c * D : (c + 1) * D],
            ],
            axis=0,
        )
        in_maps.append(
            {
                "xT": xT,
                "wT": np.ascontiguousarray(w_c.T).astype(bf16),
                "cs": cs,
                "maskb": mbs,
                "woT": woT,
            }
        )

    import time as _time

    _t0 = _time.time()
    res = run_bass_kernel_spmd(nc, in_maps, list(range(CORES)), trace=_trace)
    global LAST_EXEC_NS
    LAST_EXEC_NS = int((_time.time() - _t0) * 1e9)
    outp = np.concatenate(
        [res.results[c]["out"] for c in range(CORES)], axis=0
    ).astype(np.float32)
    if _trace:
        return outp, res
    return outp


# revision 43
# speedup vs baseline: 24043.7138x; 24043.7138x over previous
"""Trainium2 Bass kernel for nn_Attention (GQA + RoPE + sliding-window mask).

Sharding: tensor-parallel over heads across 8 cores. Each core gets 4 q heads
and exactly 1 kv head (32 q / 8 kv heads, GQA group = 4). The reference's
quirky output flatten ((H,S,D)->(H,D,S)->reshape(S, H*D)) makes the final
projection contract over (d-parity, sequence) instead of heads, so the final
output is row-sharded by head block: core c produces rows [256c, 256c+256) of
the (2048, 4096) result with NO collective at all.

Per-core pipeline (all on one NeuronCore, same program on all 8 = pure SPMD):
  phase 1: QKV projections (fp32r matmuls) + RoPE (+fold sqrt(scale) into the
           rope tables of both q and k) + PE transposes into [d, s] layouts.
  phase 2: per (head, 512-query-super): scores (fp32r), 2-pass masked softmax
           (DVE max / ACT fused exp+sum), PE-transpose P to [k, q] (bf16),
           PV matmul (bf16) -> A^T, transpose back, normalize.
  phase 3: final projection vs full wo (bf16), row slice out.
"""

import numpy as np
from contextlib import ExitStack

P = 128
D = 128  # head dim
NH = 4   # q heads per core
CORES = 8
NEG_THRESH = -1e8


def _dtypes():
    import concourse.mybir as mybir

    return mybir


def build_attention_nc(
    SEQ,
    DIM,
    plan,
    n_uniq,
    p_dt_name="bfloat16",
    wo_dt_name="bfloat16",
    proj_dt_name="bfloat16",
    proj_f32r=True,
    score_f32r=True,
    use_dma_t=True,
):
    """Build the per-core Bass program.

    plan: list over q-tiles i (SEQ//128 entries) of lists of (chunk_idx, uid)
          where uid == -1 means the 512-wide chunk needs no mask add, else the
          index into the maskb tensor. Chunks absent from the list are fully
          masked (skipped).
    """
    import concourse.bass as bass
    import concourse.bacc as bacc
    import concourse.mybir as mybir
    import concourse.tile as tile
    from concourse.masks import make_identity

    f32 = mybir.dt.float32
    f32r = mybir.dt.float32r
    P_DT = getattr(mybir.dt, p_dt_name)
    WO_DT = getattr(mybir.dt, wo_dt_name)
    PJ_DT = getattr(mybir.dt, proj_dt_name)
    pj_f32r = proj_f32r and proj_dt_name == "float32"

    ST = SEQ // P          # 16 s-tiles
    DD = DIM // P          # 32 contraction tiles
    KC = SEQ // 512        # 4 key chunks
    QS = SEQ // 512        # 4 query supers
    EW = NH * D            # 512 q-projection width
    JT = 2 * SEQ // P      # 32 j-tiles for final matmul
    MC = DIM // 512        # 8 output chunks
    ITILES = (NH * 64) // P  # 2 output row tiles
    assert NH == 4 and SEQ % 512 == 0 and DIM % 512 == 0

    def mm_cast(ap, use_r):
        return ap.bitcast(f32r) if use_r else ap

    nc = bacc.Bacc(trn_type="TRN2", debug=False, num_devices=CORES)

    # x pre-tiled on host: xT[p, st, t, si] = x[st*128+si, t*128+p] so each
    # streamed chunk is one DMA with 2KB contiguous per-partition runs
    xT = nc.dram_tensor("xT", [P, ST, DD, P], PJ_DT, kind="ExternalInput").ap()
    wT = nc.dram_tensor("wT", [DIM, EW + 2 * D], PJ_DT, kind="ExternalInput").ap()
    cs = nc.dram_tensor("cs", [SEQ, EW], f32, kind="ExternalInput").ap()
    mb = nc.dram_tensor(
        "maskb", [max(n_uniq, 1), P, 512], f32, kind="ExternalInput"
    ).ap()
    woT = nc.dram_tensor("woT", [2 * SEQ, DIM], WO_DT, kind="ExternalInput").ap()
    out = nc.dram_tensor("out", [NH * 64, DIM], f32, kind="ExternalOutput").ap()

    with tile.TileContext(nc) as tc, ExitStack() as ctx:
        const = ctx.enter_context(tc.tile_pool(name="const", bufs=1))
        idF = const.tile([P, P], f32)
        make_identity(nc, idF)
        idP = const.tile([P, P], P_DT)
        make_identity(nc, idP)
        zeros = const.tile([P, 512], f32)
        nc.vector.memset(zeros, 0.0)

        pers = ctx.enter_context(tc.tile_pool(name="pers", bufs=1))
        QTt = pers.tile([P, NH, ST * P], f32)   # [d, h, s]
        KTt = pers.tile([P, ST * P], f32)       # [d, s]
        Vt = pers.tile([P, ST, D], P_DT)        # [k(part), ktile, d]
        if n_uniq > 0:
            mbt = pers.tile([P, n_uniq, 512], f32)

        # ---------------- phase 1: projections + rope + layout ----------------
        with (
            tc.tile_pool(name="wpool", bufs=1) as wpool,
            tc.tile_pool(name="xpool", bufs=6) as xpool,
            tc.tile_pool(name="cspool", bufs=2) as cspool,
            tc.tile_pool(name="rpool", bufs=2) as rpool,
            tc.tile_pool(name="qps", bufs=2, space="PSUM") as qps,
            tc.tile_pool(name="kvps", bufs=2, space="PSUM") as kvps,
            tc.tile_pool(name="tps", bufs=2, space="PSUM") as tps,
            tc.tile_pool(name="t2ps", bufs=2, space="PSUM") as t2ps,
        ):
            XGW = min(8, DD)
            wTt = wpool.tile([P, DD, EW + 2 * D], PJ_DT)
            wTr = wT.rearrange("(t p) e -> p t e", p=P)

            XG = min(8, DD)  # dd-tiles per streamed x chunk
            NG = DD // XG
            xTr = xT
            # Interleave the weight-chunk loads with s-tile 0's x chunks so
            # the first matmuls start as soon as chunk 0 of each lands.
            st0_x = []
            for g in range(NG):
                xTt = xpool.tile([P, XG, P], PJ_DT, tag="xT")
                nc.sync.dma_start(
                    out=xTt, in_=xTr[:, 0, g * XG : (g + 1) * XG, :]
                )
                st0_x.append(xTt)
                gw = g % (DD // XGW)
                nc.sync.dma_start(
                    out=wTt[:, gw * XGW : (gw + 1) * XGW, :],
                    in_=wTr[:, gw * XGW : (gw + 1) * XGW, :],
                )
            for st in range(ST):
                cst = cspool.tile([P, EW], f32, tag="cs")
                nc.sync.dma_start(out=cst, in_=cs[st * P : (st + 1) * P, :])

                Qp = qps.tile([P, EW], f32, tag="Qp")
                KVp = kvps.tile([P, 2 * D], f32, tag="KVp")
                for g in range(DD // XG):
                    if st == 0:
                        xTt = st0_x[g]
                    else:
                        xTt = xpool.tile([P, XG, P], PJ_DT, tag="xT")
                        nc.sync.dma_start(
                            out=xTt,
                            in_=xTr[:, st, g * XG : (g + 1) * XG, :],
                        )
                    for tt in range(XG):
                        t = g * XG + tt
                        lhsT = mm_cast(xTt[:, tt, :], pj_f32r)
                        nc.tensor.matmul(
                            Qp,
                            lhsT,
                            mm_cast(wTt[:, t, 0:EW], pj_f32r),
                            start=(t == 0),
                            stop=(t == DD - 1),
                        )
                        nc.tensor.matmul(
                            KVp,
                            lhsT,
                            mm_cast(wTt[:, t, EW : EW + 2 * D], pj_f32r),
                            start=(t == 0),
                            stop=(t == DD - 1),
                        )

                # rope via strided even/odd halves (2-level APs only — 3-level
                # APs overflow the fixed ISA instruction encoding).
                # tensor_tensor_reduce instead of tensor_tensor: the plain TT
                # ISA struct has a single sync-wait slot and walrus codegen
                # rejects the PE+DMA double wait Tile emits here; the TTR/ISA
                # struct carries up to 8. accum outputs are dummies.
                def ttr_ew(out, in0, in1, op):
                    nc.vector.tensor_tensor(out=out, in0=in0, in1=in1, op=op)

                A_ = mybir.AluOpType
                HF = EW // 2  # 256: cos table width for q
                rq = rpool.tile([P, EW], f32, tag="rq")
                t1 = rpool.tile([P, HF], f32, tag="t1")
                t2 = rpool.tile([P, HF], f32, tag="t2")
                q_ev, q_od = Qp[:, 0:EW:2], Qp[:, 1:EW:2]
                cosr, sinr = cst[:, 0:HF], cst[:, HF : 2 * HF]
                ttr_ew(t1, q_ev, cosr, A_.mult)
                ttr_ew(t2, q_od, sinr, A_.mult)
                ttr_ew(rq[:, 0:EW:2], t1, t2, A_.subtract)
                ttr_ew(t1, q_ev, sinr, A_.mult)
                ttr_ew(t2, q_od, cosr, A_.mult)
                ttr_ew(rq[:, 1:EW:2], t1, t2, A_.add)

                rk = rpool.tile([P, D], f32, tag="rk")
                k_ev, k_od = KVp[:, 0:D:2], KVp[:, 1:D:2]
                cosk, sink = cst[:, 0 : D // 2], cst[:, HF : HF + D // 2]
                ttr_ew(t1[:, 0 : D // 2], k_ev, cosk, A_.mult)
                ttr_ew(t2[:, 0 : D // 2], k_od, sink, A_.mult)
                ttr_ew(rk[:, 0:D:2], t1[:, 0 : D // 2], t2[:, 0 : D // 2], A_.subtract)
                ttr_ew(t1[:, 0 : D // 2], k_ev, sink, A_.mult)
                ttr_ew(t2[:, 0 : D // 2], k_od, cosk, A_.mult)
                ttr_ew(rk[:, 1:D:2], t1[:, 0 : D // 2], t2[:, 0 : D // 2], A_.add)

                # V -> bf16 [k, d] layout (ACT copy, cast)
                nc.scalar.activation(
                    out=Vt[:, st, :],
                    in_=KVp[:, D : 2 * D],
                    func=mybir.ActivationFunctionType.Copy,
                )

                # transpose rq (per head) and rk into [d, s] layouts
                T1 = tps.tile([P, EW], f32, tag="T1")
                for h in range(NH):
                    nc.tensor.transpose(
                        T1[:, h * P : (h + 1) * P], rq[:, h * P : (h + 1) * P], idF
                    )
                # write as f32r so walrus accepts them as f32r matmul operands
                nc.vector.tensor_copy(
                    out=mm_cast(QTt[:, :, st * P : (st + 1) * P], score_f32r),
                    in_=T1.rearrange("p (h s) -> p h s", h=NH),
                )
                T2 = t2ps.tile([P, P], f32, tag="T2")
                nc.tensor.transpose(T2, rk, idF)
                nc.vector.tensor_copy(
                    out=mm_cast(KTt[:, st * P : (st + 1) * P], score_f32r), in_=T2
                )

        # ---------------- phase 2: attention ----------------
        if n_uniq > 0:
            nc.sync.dma_start(out=mbt, in_=mb.rearrange("u p m -> p u m"))
        apool = ctx.enter_context(tc.tile_pool(name="apool", bufs=1))
        # split by head-pair so phase 3's first row-tile can start once
        # heads 0-1 finish, overlapping the rest of phase 2
        Aall = [
            apool.tile([P, 2 * ST * D], P_DT, name=f"Aall{i}")
            for i in range(NH // 2)
        ]
        with (
            tc.tile_pool(name="ptsb", bufs=2) as ptsb,
            tc.tile_pool(name="spool", bufs=6) as spool,
            tc.tile_pool(name="ppool", bufs=4) as ppool,
            tc.tile_pool(name="stat", bufs=12) as stat,
            tc.tile_pool(name="atsb", bufs=3) as atsb,
            tc.tile_pool(name="sps", bufs=2, space="PSUM") as sps,
            tc.tile_pool(name="ptps", bufs=2, space="PSUM") as ptps,
            tc.tile_pool(name="atps", bufs=1, space="PSUM") as atps,
            tc.tile_pool(name="aps", bufs=1, space="PSUM") as aps,
            tc.tile_pool(name="wopool", bufs=2) as wopool,
            tc.tile_pool(name="osb", bufs=2) as osb,
            tc.tile_pool(name="ops", bufs=2, space="PSUM") as ops,
        ):
            for h in range(NH):
                for qs in range(QS):
                    PTt = ptsb.tile([P, ST, 512], P_DT, tag="PT")
                    kts_used = set()
                    recips = []
                    pt_written = set()
                    for qi in range(4):
                        i = 4 * qs + qi
                        row = plan[i]
                        if not row:
                            recips.append(None)
                            continue
                        pairs = [row[k : k + 2] for k in range(0, len(row), 2)]
                        stats = stat.tile([P, KC], f32, tag="stats")
                        ncols = 0
                        S_tiles = []
                        for pr in pairs:
                            W = 512 * len(pr)
                            S = sps.tile([P, 1024], f32, tag="S")
                            masked_any = any(uid >= 0 for (_, uid) in pr)
                            for k, (c, uid) in enumerate(pr):
                                sl = S[:, k * 512 : (k + 1) * 512]
                                nc.tensor.matmul(
                                    sl,
                                    mm_cast(
                                        QTt[:, h, i * P : (i + 1) * P], score_f32r
                                    ),
                                    mm_cast(
                                        KTt[:, c * 512 : (c + 1) * 512], score_f32r
                                    ),
                                    start=True,
                                    stop=True,
                                )
                                if uid >= 0:
                                    nc.vector.tensor_add(sl, sl, mbt[:, uid, :])
                                if masked_any or len(pr) == 1:
                                    nc.vector.tensor_reduce(
                                        out=stats[:, ncols : ncols + 1],
                                        in_=sl,
                                        axis=mybir.AxisListType.X,
                                        op=mybir.AluOpType.max,
                                    )
                                    ncols += 1
                            if not masked_any and len(pr) == 2:
                                # one pair-wide max over both chunks
                                nc.vector.tensor_reduce(
                                    out=stats[:, ncols : ncols + 1],
                                    in_=S,
                                    axis=mybir.AxisListType.X,
                                    op=mybir.AluOpType.max,
                                )
                                ncols += 1
                            S_tiles.append((S, pr))
                        negm = stat.tile([P, 1], f32, tag="negm")
                        nc.vector.tensor_reduce(
                            out=negm,
                            in_=stats[:, 0:ncols],
                            axis=mybir.AxisListType.X,
                            op=mybir.AluOpType.max,
                            negate=True,
                        )
                        sums = stat.tile([P, KC], f32, tag="sums")
                        for k, (Sk, pr) in enumerate(S_tiles):
                            W = 512 * len(pr)
                            Pt = ppool.tile([P, 1024], P_DT, tag="P")
                            nc.scalar.activation(
                                out=Pt[:, 0:W],
                                in_=Sk[:, 0:W],
                                func=mybir.ActivationFunctionType.Exp,
                                bias=negm,
                                accum_out=sums[:, k : k + 1],
                            )
                            # transpose P [q, k] -> PT [k, q]
                            for j, (c, uid) in enumerate(pr):
                                if use_dma_t:
                                    nc.sync.dma_start_transpose(
                                        out=PTt[
                                            :, 4 * c : 4 * c + 4, qi * P : (qi + 1) * P
                                        ],
                                        in_=Pt[:, j * 512 : (j + 1) * 512],
                                    )
                                else:
                                    PTp = ptps.tile([P, 512], P_DT, tag="PTp")
                                    for jj in range(4):
                                        nc.tensor.transpose(
                                            PTp[:, jj * P : (jj + 1) * P],
                                            Pt[:, j * 512 + jj * P : j * 512 + (jj + 1) * P],
                                            idP,
                                        )
                                    nc.vector.tensor_copy(
                                        out=PTt[:, 4 * c : 4 * c + 4, qi * P : (qi + 1) * P],
                                        in_=PTp.rearrange("p (kt q) -> p kt q", kt=4),
                                    )
                                for jj in range(4):
                                    kts_used.add(4 * c + jj)
                                    pt_written.add((4 * c + jj, qi))
                        denom = stat.tile([P, 1], f32, tag="denom")
                        nc.vector.tensor_reduce(
                            out=denom,
                            in_=sums[:, 0 : len(S_tiles)],
                            axis=mybir.AxisListType.X,
                            op=mybir.AluOpType.add,
                        )
                        recip = stat.tile([P, 1], f32, tag="recip")
                        nc.vector.reciprocal(recip, denom)
                        recips.append(recip)

                    # zero-fill PT holes (only for non-causal masks)
                    kts = sorted(kts_used)
                    for kt in kts:
                        for qi in range(4):
                            if (kt, qi) not in pt_written and recips[qi] is not None:
                                nc.vector.memset(
                                    PTt[:, kt, qi * P : (qi + 1) * P], 0.0
                                )
                            elif recips[qi] is None:
                                nc.vector.memset(
                                    PTt[:, kt, qi * P : (qi + 1) * P], 0.0
                                )

                    if not kts:
                        continue
                    # PV: A^T[d, q] accumulated over key tiles
                    At = atps.tile([P, 512], f32, tag="At")
                    for n, kt in enumerate(kts):
                        nc.tensor.matmul(
                            At,
                            Vt[:, kt, :],
                            PTt[:, kt, :],
                            start=(n == 0),
                            stop=(n == len(kts) - 1),
                        )
                    Atsb = atsb.tile([P, 512], P_DT, tag="Atsb")
                    nc.vector.tensor_copy(out=Atsb, in_=At)
                    Ap = aps.tile([P, 512], P_DT, tag="Ap")
                    for qi in range(4):
                        nc.tensor.transpose(
                            Ap[:, qi * P : (qi + 1) * P],
                            Atsb[:, qi * P : (qi + 1) * P],
                            idP,
                        )
                    # Aall layout: [sp, (t*2 + dd)*128 + hb*64 + p] so the final
                    # matmul's stationary slices are contiguous (walrus requires
                    # a single free dim on weight APs)
                    Ah = Aall[h // 2]
                    hb = h % 2
                    for qi in range(4):
                        i = 4 * qs + qi
                        # dview[sp, p, dd] == Ah[:, i*256 + dd*128 + hb*64 + p]
                        dview = Ah[:, i * 2 * P : (i + 1) * 2 * P].rearrange(
                            "a (dd j) -> a dd j", dd=2
                        )[:, :, hb * 64 : hb * 64 + 64].rearrange(
                            "a dd p -> a p dd"
                        )
                        if recips[qi] is None:
                            nc.vector.memset(dview, 0.0)
                            continue
                        nc.scalar.activation(
                            out=dview,
                            in_=Ap[:, qi * P : (qi + 1) * P].rearrange(
                                "a (p two) -> a p two", two=2
                            ),
                            func=mybir.ActivationFunctionType.Copy,
                            scale=recips[qi],
                        )

            # ---------------- phase 3: output projection ----------------
            for mc in range(MC):
                wot = wopool.tile([P, JT, 512], WO_DT, tag="wo")
                nc.sync.dma_start(
                    out=wot,
                    in_=woT[:, mc * 512 : (mc + 1) * 512].rearrange(
                        "(t p) m -> p t m", p=P
                    ),
                )
                for it in range(ITILES):
                    O = ops.tile([P, 512], f32, tag="O")
                    Av = Aall[it]
                    for jt in range(JT):
                        ddj, t = jt // ST, jt % ST
                        lhsT = Av[:, (t * 2 + ddj) * P : (t * 2 + ddj + 1) * P]
                        nc.tensor.matmul(
                            O,
                            lhsT,
                            wot[:, jt, :],
                            start=(jt == 0),
                            stop=(jt == JT - 1),
                        )
                    Ot = osb.tile([P, 512], f32, tag="Ot")
                    nc.scalar.activation(
                        out=Ot, in_=O, func=mybir.ActivationFunctionType.Copy
                    )
                    nc.sync.dma_start(
                        out=out[it * P : (it + 1) * P, mc * 512 : (mc + 1) * 512],
                        in_=Ot,
                    )

    # Bacc.compile() legalizes sync (>=2 waits split into EventSemaphore
    # instructions — this walrus caps every instruction at ONE sync wait)
    nc.compile()
    return nc


def analyze_mask(mask, SEQ):
    """Classify 128x512 mask blocks: skip / free / masked(dedup uid)."""
    ST = SEQ // P
    KC = SEQ // 512
    uniq = {}
    blocks = []
    plan = []
    for i in range(ST):
        row = []
        for c in range(KC):
            blk = mask[i * P : (i + 1) * P, c * 512 : (c + 1) * 512]
            if (blk <= NEG_THRESH).all():
                continue
            if not blk.any():
                row.append((c, -1))
            else:
                key = blk.tobytes()
                if key not in uniq:
                    uniq[key] = len(blocks)
                    blocks.append(np.ascontiguousarray(blk))
                row.append((c, uniq[key]))
        if not row:
            # fully masked query rows: keep all chunks so softmax matches
            # the reference's uniform distribution over -1e9 logits
            for c in range(KC):
                blk = mask[i * P : (i + 1) * P, c * 512 : (c + 1) * 512]
                key = blk.tobytes()
                if key not in uniq:
                    uniq[key] = len(blocks)
                    blocks.append(np.ascontiguousarray(blk))
                row.append((c, uniq[key]))
        plan.append(row)
    return plan, blocks


def make_rope_tables(cos_freq, sin_freq, SEQ, scale_quarter):
    """Build replicated [cos2 | sin2] tables with sqrt(SCALE) folded in.

    [cos_rep (SEQ, NH*64) | sin_rep (SEQ, NH*64)], sqrt(scale) folded in
    """
    cos_t = np.tile(np.asarray(cos_freq, np.float32) * scale_quarter, (1, NH))
    sin_t = np.tile(np.asarray(sin_freq, np.float32) * scale_quarter, (1, NH))
    return np.ascontiguousarray(
        np.concatenate([cos_t, sin_t], axis=1).astype(np.float32)
    )


_BUILD_CACHE = {}


def kernel(
    x,
    cos_freq,
    sin_freq,
    positions,
    mask,
    wq,
    wk,
    wv,
    wo,
    _trace=False,
):
    import sys

    if "/opt/trn_rl_repo" not in sys.path:
        sys.path.insert(0, "/opt/trn_rl_repo")
    from concourse.bass_utils import run_bass_kernel_spmd

    x = np.asarray(x, np.float32)
    mask = np.asarray(mask, np.float32)
    wq = np.asarray(wq, np.float32)
    wk = np.asarray(wk, np.float32)
    wv = np.asarray(wv, np.float32)
    wo = np.asarray(wo, np.float32)
    SEQ, DIM = x.shape
    assert wq.shape[0] == CORES * NH * D and wk.shape[0] == CORES * D
    assert 2 * SEQ == wq.shape[0], "flatten structure requires H*D == 2*SEQ"

    plan, blocks = analyze_mask(mask, SEQ)
    n_uniq = len(blocks)
    key = (SEQ, DIM, tuple(tuple(r) for r in plan))
    if key not in _BUILD_CACHE:
        _BUILD_CACHE[key] = build_attention_nc(SEQ, DIM, plan, n_uniq)
    nc = _BUILD_CACHE[key]

    import ml_dtypes

    bf16 = ml_dtypes.bfloat16
    scale_quarter = np.float32(D ** -0.25)
    cs = make_rope_tables(cos_freq, sin_freq, SEQ, scale_quarter)
    ST_, DD_ = SEQ // P, DIM // P
    xT = np.ascontiguousarray(
        x.reshape(ST_, P, DD_, P).transpose(3, 0, 2, 1)
    ).astype(bf16)
    woT = np.ascontiguousarray(wo.T).astype(bf16)
    if n_uniq:
        mbs = np.ascontiguousarray(np.stack(blocks, axis=0))
    else:
        mbs = np.zeros((1, P, 512), np.float32)

    in_maps = []
    for c in range(CORES):
        w_c = np.concatenate(
            [
                wq[c * NH * D : (c + 1) * NH * D],
                wk[c * D : (c + 1) * D],
                wv[c * D : (c + 1) * D],
            ],
            axis=0,
        )
        in_maps.append(
            {
                "xT": xT,
                "wT": np.ascontiguousarray(w_c.T).astype(bf16),
                "cs": cs,
                "maskb": mbs,
                "woT": woT,
            }
        )

    import time as _time

    _t0 = _time.time()
    res = run_bass_kernel_spmd(nc, in_maps, list(range(CORES)), trace=_trace)
    global LAST_EXEC_NS
    LAST_EXEC_NS = int((_time.time() - _t0) * 1e9)
    outp = np.concatenate(
        [res.results[c]["out"] for c in range(CORES)], axis=0
    ).astype(np.float32)
    if _trace:
        return outp, res
    return outp
